# revision 18
# baseline (speedup 1.0000x reference)
"""BiGCN (two-branch GCN + root-extend + scatter-mean + MLP) on 8 trn2
NeuronCores — single SPMD launch with cooperative table builds.

Sharding: global 128-node tiles are partitioned across cores by graph
boundaries (core c owns tiles [t0[c], t0[c+1])). Each conv layer builds its
normalized message table ht = dinv * (act @ W) cooperatively: every core
computes only its owned tiles, then an AllGather replicates the full
node-major table to all cores. Aggregation (per-dst-tile indirect-DMA gather
of ht[src] + one-hot PE matmul segment-sum, self-loops as real edges) is
dst-sharded by the same windows, so scatter-mean pooling stays core-local.
Layer-2 table tiles are built inside layer-1's consume (fused transpose +
relu + matmul + root-row gather), and root features x2[root_g] are extracted
with a one-hot accumulation matmul, so no host round-trip is needed anywhere:
one launch does both layers + pooling + MLP.
"""
import numpy as np

import concourse.bacc as bacc
import concourse.mybir as mybir
import concourse.tile as tile
from concourse.bass_utils import run_bass_kernel_spmd

P = 128
N_CORES = 8
F32 = mybir.dt.float32
I16 = mybir.dt.int16

# table/message storage dtype (PSUM accumulation is always fp32)
TBL_DT = mybir.dt.bfloat16
TBL_NP = np.float32

CHK = 32768  # dma_gather table-chunk rows (int16 index range)


# ----------------------------------------------------------------------------
# host-side preprocessing (index manipulation only)
# ----------------------------------------------------------------------------

def _ceil(a, b):
    return -(-a // b)


def _edges_for_core(es_g, d0, gid_all, lo, hi, T, NQ):
    """Edges with dst in [lo, hi) plus one self-edge per real window node.
    src already remapped to gathered-table row ids (gid). Sorted by
    (dst tile, src); per-(tile, src-chunk) counts."""
    m = (d0 >= lo) & (d0 < hi)
    es = es_g[m]
    ed = (d0[m] - lo).astype(np.int64)
    sl = np.arange(lo, hi, dtype=np.int64)
    es = np.concatenate([es, gid_all[sl]])
    ed = np.concatenate([ed, sl - lo])
    tl = ed >> 7
    order = np.lexsort((es, tl))
    es, ed, tl = es[order], ed[order], tl[order]
    q = es >> 15
    cnt_tq = np.bincount(tl * NQ + q, minlength=T * NQ).reshape(T, NQ)
    return es, ed, tl, q, cnt_tq


def _pack_edges(branch_cores, T, NQ):
    """Union-max per-(tile, chunk) block counts sb[t][q]; per-core padded
    arrays: IDX16 [128, Mbar*8] int16 (dma_gather wrapped layout, idx
    relative to chunk, pad=0) and DSTL [128, Mbar] f32 (pad=-1). Flat edge
    slot j of segment (t,q) at block boff[t][q]+j//128, partition j%128 —
    exactly dma_gather's output layout."""
    sb = np.stack([(c["cnt_tq"] + P - 1) // P for c in branch_cores]).max(axis=0)
    boff = np.concatenate([[0], np.cumsum(sb.ravel())]).reshape(-1)[:-1].reshape(T, NQ)
    mb = sb.sum(axis=1)
    off = np.concatenate([[0], np.cumsum(mb)])
    Mbar = max(1, int(off[-1]))
    out = []
    for c in branch_cores:
        F = np.zeros(Mbar * P, np.int16)
        DSTL = np.full((P, Mbar), -1.0, np.float32)
        es, ed, tl, q, cnt_tq = (c["es"], c["ed"], c["tl"], c["q"], c["cnt_tq"])
        if len(ed):
            segid = tl * NQ + q
            starts = np.concatenate([[0], np.cumsum(cnt_tq.ravel())])
            within = np.arange(len(ed)) - starts[segid]
            flat = boff.ravel()[segid] * P + within
            F[flat] = (es & (CHK - 1)).astype(np.int16)
            DSTL[flat & 127, flat >> 7] = (ed - (tl << 7)).astype(np.float32)
        IDX16 = np.ascontiguousarray(np.tile(F.reshape(-1, 16).T, (8, 1)))
        out.append({"IDX16": IDX16, "DSTL": DSTL})
    return (sb.astype(int), boff.astype(int), mb.astype(int).tolist(),
            off.astype(int), Mbar, out)


def _wrap16(vec):
    """dma_gather wrapped int16 index layout: [128, len/16]."""
    return np.ascontiguousarray(np.tile(vec.reshape(-1, 16).T, (8, 1)))


def preprocess(x, x_da, edge_index, batch, rootindex):
    N = x.shape[0]
    B = rootindex.shape[0]
    x0 = np.concatenate([x, x_da], axis=1).astype(np.float32)
    assert x0.shape[1] == P
    TBL = _ceil(N, P)
    batch = batch.astype(np.int64)
    rootindex = np.asarray(rootindex).astype(np.int64)

    ns = np.searchsorted(batch, np.arange(B + 1))
    g0 = [int(_ceil(B * c, N_CORES)) for c in range(N_CORES + 1)]
    t0 = [int(ns[g0[c]]) // P for c in range(N_CORES)] + [TBL]
    TO = max(t0[c + 1] - t0[c] for c in range(N_CORES))
    Tc = [(_ceil(int(ns[g0[c + 1]]), P) - t0[c]) if c < N_CORES - 1
          else (TBL - t0[c]) for c in range(N_CORES)]
    T = max(Tc)
    assert T >= TO
    G_LOC = max(g0[c + 1] - g0[c] for c in range(N_CORES))
    assert G_LOC <= P
    GTOT = N_CORES * TO * P
    NQ = _ceil(GTOT, CHK)

    # node -> gathered-table row id
    bounds = np.array([t0[c] * P for c in range(1, N_CORES)])
    own = np.searchsorted(bounds, np.arange(N), side="right")
    t0v = np.array(t0[:N_CORES])
    gid_all = own * (TO * P) + np.arange(N) - t0v[own] * P

    # full transposed features, then per-core owned slices
    x0T = np.zeros((P, TBL * P), np.float32)
    x0T[:, :N] = x0.T
    x0T_own = []
    for c in range(N_CORES):
        sl = np.zeros((P, TO * P), np.float32)
        w = min(TO * P, TBL * P - t0[c] * P)
        sl[:, :w] = x0T[:, t0[c] * P: t0[c] * P + w]
        x0T_own.append(sl)

    src_g = edge_index[0].astype(np.int64)
    dst_g = edge_index[1].astype(np.int64)

    branches = {}
    for name, (s0, d0) in {"td": (src_g, dst_g), "bu": (dst_g, src_g)}.items():
        deg = (np.bincount(d0, minlength=N) + 1.0).astype(np.float32)
        dinv = (1.0 / np.sqrt(deg)).astype(np.float32)
        es_g = gid_all[s0]
        cores = []
        for c in range(N_CORES):
            lo, hi = t0[c] * P, min((t0[c] + T) * P, N)
            es, ed, tl, q, cnt_tq = _edges_for_core(es_g, d0, gid_all, lo, hi,
                                                    T, NQ)
            cores.append({"es": es, "ed": ed, "tl": tl, "q": q,
                          "cnt_tq": cnt_tq})
        sb, boff, mb, off, Mbar, packed = _pack_edges(cores, T, NQ)
        bd = {"sb": sb, "boff": boff, "mbar": mb, "off": off,
              "Mbar": Mbar, "packed": packed}
        dv_own, dv_loc = [], []
        for c in range(N_CORES):
            rows = t0[c] * P + np.arange(TO * P)
            valid = rows < N
            rr = np.minimum(rows, N - 1)
            dv = np.where(valid, dinv[rr], 1.0).astype(np.float32)
            dv_own.append(np.ascontiguousarray(dv.reshape(TO, P).T))
            rows = t0[c] * P + np.arange(T * P)
            valid = rows < N
            rr = np.minimum(rows, N - 1)
            dv = np.where(valid, dinv[rr], 1.0).astype(np.float32)
            dv_loc.append(np.ascontiguousarray(dv.reshape(T, P).T))
        bd["dinv_own"] = dv_own
        bd["dinv_loc"] = dv_loc
        branches[name] = bd

    GL, GLR, cinv, NG = [], [], [], []
    for c in range(N_CORES):
        rows = t0[c] * P + np.arange(T * P)
        valid = rows < N
        rr = np.minimum(rows, N - 1)
        gl = np.where(valid, batch[rr] - g0[c], -1).astype(np.int64)
        gl = np.where((gl >= 0) & (gl < G_LOC), gl, -1).astype(np.float32)
        GL.append(np.ascontiguousarray(gl.reshape(T, P).T))
        glr = np.full(T * P, -1.0, np.float32)
        for j in range(g0[c + 1] - g0[c]):
            r = int(rootindex[g0[c] + j])
            loc = r - t0[c] * P
            assert 0 <= loc < T * P
            glr[loc] = float(j)
        GLR.append(np.ascontiguousarray(glr.reshape(T, P).T))
        cnts = np.ones(P, np.float32)
        for j in range(G_LOC):
            g = g0[c] + j
            if g < g0[c + 1]:
                cc = float(ns[g + 1] - ns[g])
                cnts[j] = cc if cc > 0 else 1.0
        cinv.append((1.0 / cnts).reshape(P, 1).astype(np.float32))
        orows = t0[c] * P + np.arange(TO * P)
        ng = np.where(orows < N, batch[np.minimum(orows, N - 1)], 0)
        NG.append(_wrap16(ng.astype(np.int16)))

    B_PAD = _ceil(B, P) * P
    rootx0T = np.zeros((P, B_PAD), np.float32)
    rootx0T[:, :B] = x0[rootindex].T

    iota = np.broadcast_to(np.arange(P, dtype=np.float32), (P, P)).copy()
    ident = np.eye(P, dtype=np.float32)

    return {"N": N, "B": B, "TBL": TBL, "B_PAD": B_PAD, "NQ": NQ,
            "T": T, "TO": TO, "G_LOC": G_LOC, "GTOT": GTOT,
            "g0": g0, "t0": t0, "x0T_own": x0T_own, "branches": branches,
            "GL": GL, "GLR": GLR, "cinv": cinv, "NG": NG,
            "rootx0T": rootx0T, "iota": iota, "ident": ident}


# ----------------------------------------------------------------------------
# device program
# ----------------------------------------------------------------------------

N_QUEUES = 4
_qctr = [0]


def _next_q():
    q = _qctr[0] % N_QUEUES
    _qctr[0] += 1
    return q


def _new_nc():
    return bacc.Bacc("TRN2", target_bir_lowering=False, debug=False,
                     num_devices=N_CORES, num_swdge_queues=N_QUEUES)


def _load(nc, pool, dram_ap, shape, dtype, tag, bufs=1):
    t = pool.tile(list(shape), dtype, tag=tag, bufs=bufs)
    nc.sync.dma_start(out=t[:], in_=dram_ap)
    return t


def _bias_tile(nc, pool, psum, ones_sb, b_sb, tag):
    """[128,128] SBUF tile holding the bias row broadcast to every partition."""
    psb = psum.tile([P, P], F32, tag="ps")
    nc.tensor.matmul(psb[:], lhsT=ones_sb[0:1, :], rhs=b_sb[0:1, :],
                     start=True, stop=True)
    bt = pool.tile([P, P], F32, tag=tag, bufs=1)
    nc.vector.tensor_copy(out=bt[:], in_=psb[:])
    return bt


def _aggregate(nc, pool, psum, table, IDX_sb, DSTL_sb, bmeta, TROWS,
               iota_sb, T, consume):
    """Per-tile segment-sum: PSUM_t = sum_e A01 . msg (self-loops are real
    edges); then consume(t, ps) finishes (scale/bias/...). Messages are
    gathered per (tile, 32k-row table chunk) with dma_gather."""
    sb, boff, mbar, off = bmeta["sb"], bmeta["boff"], bmeta["mbar"], bmeta["off"]
    NQ = sb.shape[1]
    mbmax = max(1, max(mbar))
    for t in range(T):
        mb = mbar[t]
        if mb == 0:
            continue  # window tail beyond N: no nodes, nothing to write
        msg = pool.tile([P, mbmax * P], TBL_DT, tag="msg", bufs=3)
        col = 0
        import os
        SPLIT = int(os.environ.get("K_SPLIT", "1"))
        for q in range(NQ):
            nb = int(sb[t][q])
            if nb == 0:
                continue
            base = q * CHK
            rows = min(CHK, TROWS - base)
            parts = min(SPLIT, nb)
            b0 = 0
            for pi in range(parts):
                nbp = (nb - b0) // (parts - pi)
                if nbp == 0:
                    continue
                nc.gpsimd.dma_gather(
                    out_ap=msg[:, (col + b0) * P: (col + b0 + nbp) * P]
                    .rearrange("p (b f) -> p b f", f=P),
                    in_ap=table[base: base + rows, :],
                    idxs_ap=IDX_sb[:, (boff[t][q] + b0) * 8:
                                   (boff[t][q] + b0 + nbp) * 8],
                    num_idxs=nbp * P, num_idxs_reg=nbp * P, elem_size=P,
                    queue_num=_next_q())
                b0 += nbp
            col += nb
        a01 = pool.tile([P, mbmax * P], TBL_DT, tag="a01", bufs=3)
        nc.vector.tensor_tensor(
            out=a01[:, : mb * P].rearrange("p (k f) -> p k f", f=P),
            in0=DSTL_sb[:, off[t]: off[t] + mb].to_broadcast([P, mb, P]),
            in1=iota_sb[:].unsqueeze(1).broadcast_to([P, mb, P]),
            op=mybir.AluOpType.is_equal,
        )
        ps = psum.tile([P, P], F32, tag="ps")
        for k in range(mb):
            nc.tensor.matmul(ps[:], lhsT=a01[:, k * P: (k + 1) * P],
                             rhs=msg[:, k * P: (k + 1) * P],
                             start=(k == 0), stop=(k == mb - 1))
        consume(t, ps)


def build_merged(pp, phases="ALL"):
    T, TO, G_LOC = pp["T"], pp["TO"], pp["G_LOC"]
    GTOT, B_PAD, NQ = pp["GTOT"], pp["B_PAD"], pp["NQ"]
    br = pp["branches"]
    nc = _new_nc()

    x0T_own = nc.dram_tensor("x0T_own", [P, TO * P], F32, kind="ExternalInput")
    iota = nc.dram_tensor("iota", [P, P], F32, kind="ExternalInput")
    ident = nc.dram_tensor("ident", [P, P], F32, kind="ExternalInput")
    ones_row = nc.dram_tensor("ones_row", [1, P], F32, kind="ExternalInput")
    rootx0T = nc.dram_tensor("rootx0T", [P, B_PAD], F32, kind="ExternalInput")
    NG = nc.dram_tensor("NG", [P, TO * 8], I16, kind="ExternalInput")
    GL = nc.dram_tensor("GL", [P, T], F32, kind="ExternalInput")
    GLR = nc.dram_tensor("GLR", [P, T], F32, kind="ExternalInput")
    cinv = nc.dram_tensor("cinv", [P, 1], F32, kind="ExternalInput")
    mlp_w1 = nc.dram_tensor("mlp_w1", [4 * P, 2 * P], F32, kind="ExternalInput")
    mlp_b1 = nc.dram_tensor("mlp_b1", [1, 2 * P], F32, kind="ExternalInput")
    mlp_w2 = nc.dram_tensor("mlp_w2", [2 * P, 2], F32, kind="ExternalInput")
    mlp_b2 = nc.dram_tensor("mlp_b2", [1, 2], F32, kind="ExternalInput")
    out = nc.dram_tensor("out", [P, 2], F32, kind="ExternalOutput")

    ins = {}
    for b in ("td", "bu"):
        M = br[b]["Mbar"]
        ins[b] = {
            "w1": nc.dram_tensor(f"w1{b}", [P, P], F32, kind="ExternalInput"),
            "b1": nc.dram_tensor(f"b1{b}", [1, P], F32, kind="ExternalInput"),
            "w2a": nc.dram_tensor(f"w2a{b}", [P, P], F32, kind="ExternalInput"),
            "w2b": nc.dram_tensor(f"w2b{b}", [P, P], F32, kind="ExternalInput"),
            "b2": nc.dram_tensor(f"b2{b}", [1, P], F32, kind="ExternalInput"),
            "dinv_own": nc.dram_tensor(f"dvo{b}", [P, TO], F32, kind="ExternalInput"),
            "dinv_loc": nc.dram_tensor(f"dvl{b}", [P, T], F32, kind="ExternalInput"),
            "IDX": nc.dram_tensor(f"IDX{b}", [P, M * 8], I16, kind="ExternalInput"),
            "DSTL": nc.dram_tensor(f"DSTL{b}", [P, M], F32, kind="ExternalInput"),
            "ht1": nc.dram_tensor(f"ht1{b}", [TO * P, P], TBL_DT, kind="Internal"),
            "TAB1": nc.dram_tensor(f"TAB1{b}", [GTOT, P], TBL_DT, kind="Internal"),
            "rtab": nc.dram_tensor(f"rtab{b}", [B_PAD, P], F32, kind="Internal"),
            "ht2": nc.dram_tensor(f"ht2{b}", [TO * P, P], TBL_DT, kind="Internal"),
            "TAB2": nc.dram_tensor(f"TAB2{b}", [GTOT, P], TBL_DT, kind="Internal"),
        }

    with tile.TileContext(nc) as tc:
        with (
            tc.tile_pool(name="sbuf", bufs=2) as pool,
            tc.tile_pool(name="cst", bufs=1) as cst,
            tc.tile_pool(name="psum", bufs=3, space="PSUM") as psum,
            tc.tile_pool(name="pps", bufs=2, space="PSUM") as pool_ps,
        ):
            iota_sb = _load(nc, cst, iota[:], (P, P), F32, "iota")
            ident_sb = _load(nc, cst, ident[:], (P, P), F32, "ident")
            ones_sb = _load(nc, cst, ones_row[:], (1, P), F32, "ones")
            NG_sb = _load(nc, cst, NG[:], (P, TO * 8), I16, "NG")
            GL_sb = _load(nc, cst, GL[:], (P, T), F32, "GL")
            GLR_sb = _load(nc, cst, GLR[:], (P, T), F32, "GLR")
            cinv_sb = _load(nc, cst, cinv[:], (P, 1), F32, "cinv")
            cs = {}
            for b in ("td", "bu"):
                cs[b] = {k: _load(nc, cst, ins[b][k][:],
                                  (P, P) if k in ("w1", "w2a", "w2b")
                                  else ((P, TO) if k == "dinv_own" else (P, T)),
                                  F32, f"{k}{b}")
                         for k in ("w1", "w2a", "w2b", "dinv_own", "dinv_loc")}

            # ---- phase A: owned ht1 tiles, both branches ----
            CH = 8
            for c0 in range(0, TO, CH):
                nb = min(CH, TO - c0)
                xt = pool.tile([P, CH * P], F32, tag="xt", bufs=2)
                nc.sync.dma_start(out=xt[:, : nb * P],
                                  in_=x0T_own[:, c0 * P: (c0 + nb) * P])
                st = {b: pool.tile([P, CH * P], TBL_DT, tag=f"st{b}", bufs=2,
                                   name=f"st{b}")
                      for b in ("td", "bu")}
                for j in range(nb):
                    for b in ("td", "bu"):
                        psx = psum.tile([P, P], F32, tag="ps")
                        nc.tensor.matmul(psx[:], lhsT=xt[:, j * P: (j + 1) * P],
                                         rhs=cs[b]["w1"][:], start=True, stop=True)
                        nc.vector.tensor_scalar(
                            out=st[b][:, j * P: (j + 1) * P], in0=psx[:],
                            scalar1=cs[b]["dinv_own"][:, c0 + j: c0 + j + 1],
                            scalar2=None, op0=mybir.AluOpType.mult)
                for b in ("td", "bu"):
                    nc.sync.dma_start(
                        out=ins[b]["ht1"][c0 * P: (c0 + nb) * P, :]
                        .rearrange("(j p) f -> p j f", p=P),
                        in_=st[b][:, : nb * P].rearrange("p (j f) -> p j f", f=P))

            # ---- phase B: AllGather ht1 -> TAB1 ----
            for b in ("td", "bu"):
                nc.gpsimd.collective_compute(
                    "AllGather", mybir.AluOpType.bypass,
                    replica_groups=[list(range(N_CORES))],
                    ins=[ins[b]["ht1"][:]], outs=[ins[b]["TAB1"][:]])

            if phases == "AB":
                nc.gpsimd.dma_start(out=out[:, :], in_=ins["td"]["TAB1"][0:P, 0:2])
            # ---- phase C: root tables R_b = relu(x0[roots]) @ W2b_b ----
            for j in range(B_PAD // P):
                rx = pool.tile([P, P], F32, tag="rx", bufs=2)
                nc.sync.dma_start(out=rx[:], in_=rootx0T[:, j * P: (j + 1) * P])
                rr = pool.tile([P, P], F32, tag="rr", bufs=2)
                nc.scalar.activation(out=rr[:], in_=rx[:],
                                     func=mybir.ActivationFunctionType.Relu)
                for b in ("td", "bu"):
                    psr = psum.tile([P, P], F32, tag="ps")
                    nc.tensor.matmul(psr[:], lhsT=rr[:], rhs=cs[b]["w2b"][:],
                                     start=True, stop=True)
                    ro = pool.tile([P, P], F32, tag="ro", bufs=2)
                    nc.vector.tensor_copy(out=ro[:], in_=psr[:])
                    nc.sync.dma_start(out=ins[b]["rtab"][j * P: (j + 1) * P, :],
                                      in_=ro[:])

            # ---- phase D: layer-1 aggregation, fused ht2-tile build ----
            rx2 = {}
            for b in ("td", "bu"):
                ib = ins[b]
                M = br[b]["Mbar"]
                IDX_sb = _load(nc, pool, ib["IDX"][:], (P, M * 8), I16, "idx")
                DSTL_sb = _load(nc, pool, ib["DSTL"][:], (P, M), F32, "dstl")
                b1_sb = _load(nc, cst, ib["b1"][:], (1, P), F32, f"b1{b}")
                btile1 = _bias_tile(nc, pool, psum, ones_sb, b1_sb, f"bt1{b}")
                ps_root = pool_ps.tile([G_LOC, P], F32, tag="acc")
                t_last = max(t for t in range(T) if br[b]["mbar"][t] > 0)
                rg_all = pool.tile([P, TO * P], F32, tag="rga", bufs=1)
                for c0 in range(0, TO, 8):
                    nb = min(8, TO - c0)
                    nc.gpsimd.dma_gather(
                        out_ap=rg_all[:, c0 * P: (c0 + nb) * P]
                        .rearrange("p (b f) -> p b f", f=P),
                        in_ap=ib["rtab"][:, :],
                        idxs_ap=NG_sb[:, c0 * 8: (c0 + nb) * 8],
                        num_idxs=nb * P, num_idxs_reg=nb * P, elem_size=P,
                        queue_num=_next_q())

                def consume1(t, ps, b=b, ib=ib, btile1=btile1, ps_root=ps_root,
                             t_last=t_last, rg_all=rg_all):
                    dvl = cs[b]["dinv_loc"]
                    xo = pool.tile([P, P], F32, tag="xo", bufs=3)
                    nc.vector.tensor_scalar(
                        out=xo[:], in0=ps[:], scalar1=dvl[:, t: t + 1],
                        scalar2=None, op0=mybir.AluOpType.mult)
                    nc.vector.tensor_add(out=xo[:], in0=xo[:], in1=btile1[:])
                    # root-feature accumulation: ps_root += onehot(GLR)^T @ xo
                    ohr = pool.tile([P, G_LOC], F32, tag="ohr", bufs=3)
                    nc.vector.tensor_tensor(
                        out=ohr[:], in0=GLR_sb[:, t: t + 1].to_broadcast([P, G_LOC]),
                        in1=iota_sb[:, :G_LOC], op=mybir.AluOpType.is_equal)
                    nc.tensor.matmul(ps_root[:], lhsT=ohr[:], rhs=xo[:],
                                     start=(t == 0), stop=(t == t_last))
                    if t < TO:
                        # fused layer-2 table tile:
                        # ht2 = dinv * (relu(x2)^T W2a + R[batch])
                        pst = psum.tile([P, P], F32, tag="pst")
                        nc.tensor.transpose(out=pst[:], in_=xo[:],
                                            identity=ident_sb[:])
                        xr = pool.tile([P, P], F32, tag="xr", bufs=3)
                        nc.scalar.activation(out=xr[:], in_=pst[:],
                                             func=mybir.ActivationFunctionType.Relu)
                        ps2 = psum.tile([P, P], F32, tag="ps2", bufs=2)
                        nc.tensor.matmul(ps2[:], lhsT=xr[:], rhs=cs[b]["w2a"][:],
                                         start=True, stop=False)
                        nc.tensor.matmul(ps2[:], lhsT=ident_sb[:],
                                         rhs=rg_all[:, t * P: (t + 1) * P],
                                         start=False, stop=True)
                        st2 = pool.tile([P, P], TBL_DT, tag="st2", bufs=3)
                        nc.vector.tensor_scalar(
                            out=st2[:], in0=ps2[:], scalar1=dvl[:, t: t + 1],
                            scalar2=None, op0=mybir.AluOpType.mult)
                        nc.sync.dma_start(
                            out=ib["ht2"][t * P: (t + 1) * P, :], in_=st2[:])

                _aggregate(nc, pool, psum, ib["TAB1"], IDX_sb, DSTL_sb,
                           br[b], GTOT, iota_sb, T, consume1)

                # finish root rows: [G_LOC, P] -> [P, G_LOC]
                rootS = pool.tile([G_LOC, P], F32, tag=f"rootS{b}", bufs=1)
                nc.vector.tensor_copy(out=rootS[:], in_=ps_root[:])
                psT = psum.tile([P, G_LOC], F32, tag="pst")
                nc.tensor.transpose(out=psT[:], in_=rootS[:],
                                    identity=ident_sb[:G_LOC, :G_LOC])
                rx2_sb = pool.tile([P, G_LOC], F32, tag=f"rx2{b}", bufs=1)
                nc.vector.tensor_copy(out=rx2_sb[:], in_=psT[:])
                rx2[b] = rx2_sb

                # ---- phase E: AllGather ht2 -> TAB2 (overlaps next branch) ----
                nc.gpsimd.collective_compute(
                    "AllGather", mybir.AluOpType.bypass,
                    replica_groups=[list(range(N_CORES))],
                    ins=[ib["ht2"][:]], outs=[ib["TAB2"][:]])

            # ---- phase F: layer-2 aggregation + pooling ----
            pooled = {}
            for b in ("td", "bu"):
                ib = ins[b]
                M = br[b]["Mbar"]
                IDX_sb = _load(nc, pool, ib["IDX"][:], (P, M * 8), I16, "idx")
                DSTL_sb = _load(nc, pool, ib["DSTL"][:], (P, M), F32, "dstl")
                b2_sb = _load(nc, cst, ib["b2"][:], (1, P), F32, f"b2{b}")
                btile2 = _bias_tile(nc, pool, psum, ones_sb, b2_sb, f"bt2{b}")
                ps_pool = pool_ps.tile([G_LOC, P], F32, tag="acc")
                t_last = max(t for t in range(T) if br[b]["mbar"][t] > 0)

                def consume2(t, ps, b=b, btile2=btile2, ps_pool=ps_pool,
                             t_last=t_last):
                    hs = pool.tile([P, P], F32, tag="hs", bufs=3)
                    nc.vector.tensor_scalar(
                        out=hs[:], in0=ps[:], scalar1=cs[b]["dinv_loc"][:, t: t + 1],
                        scalar2=None, op0=mybir.AluOpType.mult)
                    nc.vector.tensor_add(out=hs[:], in0=hs[:], in1=btile2[:])
                    h2 = pool.tile([P, P], F32, tag="h2", bufs=3)
                    nc.scalar.activation(out=h2[:], in_=hs[:],
                                         func=mybir.ActivationFunctionType.Relu)
                    oh = pool.tile([P, G_LOC], F32, tag="oh", bufs=3)
                    nc.vector.tensor_tensor(
                        out=oh[:], in0=GL_sb[:, t: t + 1].to_broadcast([P, G_LOC]),
                        in1=iota_sb[:, :G_LOC], op=mybir.AluOpType.is_equal)
                    nc.tensor.matmul(ps_pool[:], lhsT=oh[:], rhs=h2[:],
                                     start=(t == 0), stop=(t == t_last))

                _aggregate(nc, pool, psum, ib["TAB2"], IDX_sb, DSTL_sb,
                           br[b], GTOT, iota_sb, T, consume2)

                meanS = pool.tile([G_LOC, P], F32, tag=f"mean{b}", bufs=1)
                nc.vector.tensor_scalar(
                    out=meanS[:], in0=ps_pool[:], scalar1=cinv_sb[:G_LOC, :],
                    scalar2=None, op0=mybir.AluOpType.mult)
                pst = psum.tile([P, G_LOC], F32, tag="pst")
                nc.tensor.transpose(out=pst[:], in_=meanS[:],
                                    identity=ident_sb[:G_LOC, :G_LOC])
                meanT = pool.tile([P, G_LOC], F32, tag=f"meanT{b}", bufs=1)
                nc.vector.tensor_copy(out=meanT[:], in_=pst[:])
                pooled[b] = meanT

            # ---- phase G: final MLP over this core's G_LOC graph slots ----
            w1c_sb = cst.tile([P, 4 * 2 * P], F32, tag="mw1", name="mw1")
            nc.sync.dma_start(
                out=w1c_sb[:].rearrange("p (c o) -> p c o", c=4),
                in_=mlp_w1[:].rearrange("(c p) o -> p c o", p=P))
            b1m_sb = _load(nc, cst, mlp_b1[:], (1, 2 * P), F32, "mb1")
            w2c_sb = cst.tile([P, 2 * 2], F32, tag="mw2", name="mw2")
            nc.sync.dma_start(
                out=w2c_sb[:].rearrange("p (c o) -> p c o", c=2),
                in_=mlp_w2[:].rearrange("(c p) o -> p c o", p=P))
            b2m_sb = _load(nc, cst, mlp_b2[:], (1, 2), F32, "mb2")

            ps1 = psum.tile([G_LOC, 2 * P], F32, tag="ps")
            chunks = [pooled["bu"][:, :G_LOC], rx2["bu"][:, :G_LOC],
                      pooled["td"][:, :G_LOC], rx2["td"][:, :G_LOC]]
            for ci, lhsT in enumerate(chunks):
                nc.tensor.matmul(ps1[:], lhsT=lhsT,
                                 rhs=w1c_sb[:, ci * 2 * P: (ci + 1) * 2 * P],
                                 start=(ci == 0), stop=False)
            nc.tensor.matmul(ps1[:], lhsT=ones_sb[0:1, :G_LOC], rhs=b1m_sb[0:1, :],
                             start=False, stop=True)
            h1 = pool.tile([G_LOC, 2 * P], F32, tag="mlph", bufs=1)
            nc.scalar.activation(out=h1[:], in_=ps1[:],
                                 func=mybir.ActivationFunctionType.Relu)
            hT = []
            for ci in range(2):
                pst2 = psum.tile([P, G_LOC], F32, tag="pst")
                nc.tensor.transpose(out=pst2[:], in_=h1[:, ci * P: (ci + 1) * P],
                                    identity=ident_sb[:G_LOC, :G_LOC])
                ht_sb = pool.tile([P, G_LOC], F32, tag=f"hT{ci}", bufs=1)
                nc.vector.tensor_copy(out=ht_sb[:], in_=pst2[:])
                hT.append(ht_sb)
            ps2 = psum.tile([G_LOC, 2], F32, tag="ps")
            for ci in range(2):
                nc.tensor.matmul(ps2[:], lhsT=hT[ci][:, :G_LOC],
                                 rhs=w2c_sb[:, ci * 2: (ci + 1) * 2],
                                 start=(ci == 0), stop=False)
            nc.tensor.matmul(ps2[:], lhsT=ones_sb[0:1, :G_LOC], rhs=b2m_sb[0:1, :],
                             start=False, stop=True)
            oo = pool.tile([G_LOC, 2], F32, tag="oo", bufs=1)
            nc.vector.tensor_copy(out=oo[:], in_=ps2[:])
            nc.sync.dma_start(out=out[:G_LOC, :], in_=oo[:])
    nc.compile()
    return nc


# ----------------------------------------------------------------------------
# in_map assembly + kernel entry
# ----------------------------------------------------------------------------

def in_maps(pp, w):
    br = pp["branches"]
    ones = np.ones((1, P), np.float32)
    maps = []
    for c in range(N_CORES):
        m = {"x0T_own": pp["x0T_own"][c], "iota": pp["iota"],
             "ident": pp["ident"], "ones_row": ones,
             "rootx0T": pp["rootx0T"], "NG": pp["NG"][c],
             "GL": pp["GL"][c], "GLR": pp["GLR"][c], "cinv": pp["cinv"][c],
             "mlp_w1": w["mlp_w1"].astype(np.float32),
             "mlp_b1": w["mlp_b1"].astype(np.float32).reshape(1, -1),
             "mlp_w2": w["mlp_w2"].astype(np.float32),
             "mlp_b2": w["mlp_b2"].astype(np.float32).reshape(1, -1)}
        for b in ("td", "bu"):
            bb = br[b]
            m[f"w1{b}"] = np.ascontiguousarray(w[f"{b}_w1"].astype(np.float32))
            m[f"b1{b}"] = w[f"{b}_b1"].astype(np.float32).reshape(1, P)
            m[f"w2a{b}"] = np.ascontiguousarray(w[f"{b}_w2"][:P].astype(np.float32))
            m[f"w2b{b}"] = np.ascontiguousarray(w[f"{b}_w2"][P:].astype(np.float32))
            m[f"b2{b}"] = w[f"{b}_b2"].astype(np.float32).reshape(1, P)
            m[f"dvo{b}"] = bb["dinv_own"][c]
            m[f"dvl{b}"] = bb["dinv_loc"][c]
            m[f"IDX{b}"] = bb["packed"][c]["IDX16"]
            m[f"DSTL{b}"] = bb["packed"][c]["DSTL"]
        maps.append(m)
    return maps


def assemble_out(pp, results):
    B = pp["B"]
    out = np.zeros((B, 2), np.float32)
    for c in range(N_CORES):
        a, b = pp["g0"][c], pp["g0"][c + 1]
        out[a:b] = results[c]["out"][: b - a]
    return out


def kernel(x, x_da, edge_index, batch, rootindex,
           td_w1, td_b1, td_w2, td_b2,
           bu_w1, bu_b1, bu_w2, bu_b2,
           mlp_w1, mlp_b1, mlp_w2, mlp_b2):
    w = {"td_w1": td_w1, "td_b1": td_b1, "td_w2": td_w2, "td_b2": td_b2,
         "bu_w1": bu_w1, "bu_b1": bu_b1, "bu_w2": bu_w2, "bu_b2": bu_b2,
         "mlp_w1": mlp_w1, "mlp_b1": mlp_b1, "mlp_w2": mlp_w2, "mlp_b2": mlp_b2}
    w = {k: np.asarray(v) for k, v in w.items()}
    pp = preprocess(np.asarray(x), np.asarray(x_da), np.asarray(edge_index),
                    np.asarray(batch), np.asarray(rootindex))
    nc = build_merged(pp)
    res = run_bass_kernel_spmd(nc, in_maps(pp, w),
                               core_ids=list(range(N_CORES))).results
    return assemble_out(pp, res)


# revision 19
# speedup vs baseline: 1.5968x; 1.5968x over previous
"""BiGCN (two-branch GCN + root-extend + scatter-mean + MLP) on 8 trn2
NeuronCores — single SPMD launch with cooperative table builds.

Sharding: global 128-node tiles are partitioned across cores by graph
boundaries (core c owns tiles [t0[c], t0[c+1])). Each conv layer builds its
normalized message table ht = dinv * (act @ W) cooperatively: every core
computes only its owned tiles, then an AllGather replicates the full
node-major table to all cores. Aggregation (per-dst-tile indirect-DMA gather
of ht[src] + one-hot PE matmul segment-sum, self-loops as real edges) is
dst-sharded by the same windows, so scatter-mean pooling stays core-local.
Layer-2 table tiles are built inside layer-1's consume (fused transpose +
relu + matmul + root-row gather), and root features x2[root_g] are extracted
with a one-hot accumulation matmul, so no host round-trip is needed anywhere:
one launch does both layers + pooling + MLP.
"""
import numpy as np

import concourse.bacc as bacc
import concourse.mybir as mybir
import concourse.tile as tile
from concourse.bass_utils import run_bass_kernel_spmd

P = 128
N_CORES = 8
F32 = mybir.dt.float32
I16 = mybir.dt.int16

# table/message storage dtype (PSUM accumulation is always fp32)
TBL_DT = mybir.dt.bfloat16
TBL_NP = np.float32

CHK = 32768  # dma_gather table-chunk rows (int16 index range)


# ----------------------------------------------------------------------------
# host-side preprocessing (index manipulation only)
# ----------------------------------------------------------------------------

def _ceil(a, b):
    return -(-a // b)


def _edges_for_core(es_g, d0, gid_all, lo, hi, T, NQ):
    """Edges with dst in [lo, hi) plus one self-edge per real window node.
    src already remapped to gathered-table row ids (gid). Sorted by
    (dst tile, src); per-(tile, src-chunk) counts."""
    m = (d0 >= lo) & (d0 < hi)
    es = es_g[m]
    ed = (d0[m] - lo).astype(np.int64)
    sl = np.arange(lo, hi, dtype=np.int64)
    es = np.concatenate([es, gid_all[sl]])
    ed = np.concatenate([ed, sl - lo])
    tl = ed >> 7
    order = np.lexsort((es, tl))
    es, ed, tl = es[order], ed[order], tl[order]
    q = es >> 15
    cnt_tq = np.bincount(tl * NQ + q, minlength=T * NQ).reshape(T, NQ)
    return es, ed, tl, q, cnt_tq


def _pack_edges(branch_cores, T, NQ):
    """Union-max per-(tile, chunk) block counts sb[t][q]; per-core padded
    arrays: IDX16 [128, Mbar*8] int16 (dma_gather wrapped layout, idx
    relative to chunk, pad=0) and DSTL [128, Mbar] f32 (pad=-1). Flat edge
    slot j of segment (t,q) at block boff[t][q]+j//128, partition j%128 —
    exactly dma_gather's output layout."""
    sb = np.stack([(c["cnt_tq"] + P - 1) // P for c in branch_cores]).max(axis=0)
    boff = np.concatenate([[0], np.cumsum(sb.ravel())]).reshape(-1)[:-1].reshape(T, NQ)
    mb = sb.sum(axis=1)
    off = np.concatenate([[0], np.cumsum(mb)])
    Mbar = max(1, int(off[-1]))
    out = []
    for c in branch_cores:
        F = np.zeros(Mbar * P, np.int16)
        DSTL = np.full((P, Mbar), -1.0, np.float32)
        es, ed, tl, q, cnt_tq = (c["es"], c["ed"], c["tl"], c["q"], c["cnt_tq"])
        if len(ed):
            segid = tl * NQ + q
            starts = np.concatenate([[0], np.cumsum(cnt_tq.ravel())])
            within = np.arange(len(ed)) - starts[segid]
            flat = boff.ravel()[segid] * P + within
            F[flat] = (es & (CHK - 1)).astype(np.int16)
            DSTL[flat & 127, flat >> 7] = (ed - (tl << 7)).astype(np.float32)
        IDX16 = np.ascontiguousarray(np.tile(F.reshape(-1, 16).T, (8, 1)))
        out.append({"IDX16": IDX16, "DSTL": DSTL})
    return (sb.astype(int), boff.astype(int), mb.astype(int).tolist(),
            off.astype(int), Mbar, out)


def _wrap16(vec):
    """dma_gather wrapped int16 index layout: [128, len/16]."""
    return np.ascontiguousarray(np.tile(vec.reshape(-1, 16).T, (8, 1)))


def preprocess(x, x_da, edge_index, batch, rootindex):
    N = x.shape[0]
    B = rootindex.shape[0]
    x0 = np.concatenate([x, x_da], axis=1).astype(np.float32)
    assert x0.shape[1] == P
    TBL = _ceil(N, P)
    batch = batch.astype(np.int64)
    rootindex = np.asarray(rootindex).astype(np.int64)

    ns = np.searchsorted(batch, np.arange(B + 1))
    g0 = [int(_ceil(B * c, N_CORES)) for c in range(N_CORES + 1)]
    t0 = [int(ns[g0[c]]) // P for c in range(N_CORES)] + [TBL]
    TO = max(t0[c + 1] - t0[c] for c in range(N_CORES))
    Tc = [(_ceil(int(ns[g0[c + 1]]), P) - t0[c]) if c < N_CORES - 1
          else (TBL - t0[c]) for c in range(N_CORES)]
    T = max(Tc)
    assert T >= TO
    G_LOC = max(g0[c + 1] - g0[c] for c in range(N_CORES))
    assert G_LOC <= P
    GTOT = N_CORES * TO * P
    NQ = _ceil(GTOT, CHK)

    # node -> gathered-table row id
    bounds = np.array([t0[c] * P for c in range(1, N_CORES)])
    own = np.searchsorted(bounds, np.arange(N), side="right")
    t0v = np.array(t0[:N_CORES])
    gid_all = own * (TO * P) + np.arange(N) - t0v[own] * P

    # full transposed features, then per-core owned slices
    x0T = np.zeros((P, TBL * P), np.float32)
    x0T[:, :N] = x0.T
    x0T_own = []
    for c in range(N_CORES):
        sl = np.zeros((P, TO * P), np.float32)
        w = min(TO * P, TBL * P - t0[c] * P)
        sl[:, :w] = x0T[:, t0[c] * P: t0[c] * P + w]
        x0T_own.append(sl)

    src_g = edge_index[0].astype(np.int64)
    dst_g = edge_index[1].astype(np.int64)

    branches = {}
    for name, (s0, d0) in {"td": (src_g, dst_g), "bu": (dst_g, src_g)}.items():
        deg = (np.bincount(d0, minlength=N) + 1.0).astype(np.float32)
        dinv = (1.0 / np.sqrt(deg)).astype(np.float32)
        es_g = gid_all[s0]
        cores = []
        for c in range(N_CORES):
            lo, hi = t0[c] * P, min((t0[c] + T) * P, N)
            es, ed, tl, q, cnt_tq = _edges_for_core(es_g, d0, gid_all, lo, hi,
                                                    T, NQ)
            cores.append({"es": es, "ed": ed, "tl": tl, "q": q,
                          "cnt_tq": cnt_tq})
        sb, boff, mb, off, Mbar, packed = _pack_edges(cores, T, NQ)
        bd = {"sb": sb, "boff": boff, "mbar": mb, "off": off,
              "Mbar": Mbar, "packed": packed}
        dv_own, dv_loc = [], []
        for c in range(N_CORES):
            rows = t0[c] * P + np.arange(TO * P)
            valid = rows < N
            rr = np.minimum(rows, N - 1)
            dv = np.where(valid, dinv[rr], 1.0).astype(np.float32)
            dv_own.append(np.ascontiguousarray(dv.reshape(TO, P).T))
            rows = t0[c] * P + np.arange(T * P)
            valid = rows < N
            rr = np.minimum(rows, N - 1)
            dv = np.where(valid, dinv[rr], 1.0).astype(np.float32)
            dv_loc.append(np.ascontiguousarray(dv.reshape(T, P).T))
        bd["dinv_own"] = dv_own
        bd["dinv_loc"] = dv_loc
        branches[name] = bd

    GL, GLR, cinv, NG = [], [], [], []
    for c in range(N_CORES):
        rows = t0[c] * P + np.arange(T * P)
        valid = rows < N
        rr = np.minimum(rows, N - 1)
        gl = np.where(valid, batch[rr] - g0[c], -1).astype(np.int64)
        gl = np.where((gl >= 0) & (gl < G_LOC), gl, -1).astype(np.float32)
        GL.append(np.ascontiguousarray(gl.reshape(T, P).T))
        glr = np.full(T * P, -1.0, np.float32)
        for j in range(g0[c + 1] - g0[c]):
            r = int(rootindex[g0[c] + j])
            loc = r - t0[c] * P
            assert 0 <= loc < T * P
            glr[loc] = float(j)
        GLR.append(np.ascontiguousarray(glr.reshape(T, P).T))
        cnts = np.ones(P, np.float32)
        for j in range(G_LOC):
            g = g0[c] + j
            if g < g0[c + 1]:
                cc = float(ns[g + 1] - ns[g])
                cnts[j] = cc if cc > 0 else 1.0
        cinv.append((1.0 / cnts).reshape(P, 1).astype(np.float32))
        orows = t0[c] * P + np.arange(TO * P)
        ng = np.where(orows < N, batch[np.minimum(orows, N - 1)], 0)
        NG.append(_wrap16(ng.astype(np.int16)))

    B_PAD = _ceil(B, P) * P
    rootx0T = np.zeros((P, B_PAD), np.float32)
    rootx0T[:, :B] = x0[rootindex].T

    iota = np.broadcast_to(np.arange(P, dtype=np.float32), (P, P)).copy()
    ident = np.eye(P, dtype=np.float32)

    return {"N": N, "B": B, "TBL": TBL, "B_PAD": B_PAD, "NQ": NQ,
            "T": T, "TO": TO, "G_LOC": G_LOC, "GTOT": GTOT,
            "g0": g0, "t0": t0, "x0T_own": x0T_own, "branches": branches,
            "GL": GL, "GLR": GLR, "cinv": cinv, "NG": NG,
            "rootx0T": rootx0T, "iota": iota, "ident": ident}


# ----------------------------------------------------------------------------
# device program
# ----------------------------------------------------------------------------

N_QUEUES = 4
_qctr = [0]


def _next_q():
    q = _qctr[0] % N_QUEUES
    _qctr[0] += 1
    return q


def _new_nc():
    return bacc.Bacc("TRN2", target_bir_lowering=False, debug=False,
                     num_devices=N_CORES, num_swdge_queues=N_QUEUES)


def _load(nc, pool, dram_ap, shape, dtype, tag, bufs=1):
    t = pool.tile(list(shape), dtype, tag=tag, bufs=bufs)
    nc.sync.dma_start(out=t[:], in_=dram_ap)
    return t


def _bias_tile(nc, pool, psum, ones_sb, b_sb, tag):
    """[128,128] SBUF tile holding the bias row broadcast to every partition."""
    psb = psum.tile([P, P], F32, tag="ps")
    nc.tensor.matmul(psb[:], lhsT=ones_sb[0:1, :], rhs=b_sb[0:1, :],
                     start=True, stop=True)
    bt = pool.tile([P, P], F32, tag=tag, bufs=1)
    nc.vector.tensor_copy(out=bt[:], in_=psb[:])
    return bt


def _aggregate(nc, pool, psum, table, IDX_sb, DSTL_sb, bmeta, TROWS,
               iota_sb, T, consume):
    """Per-tile segment-sum: PSUM_t = sum_e A01 . msg (self-loops are real
    edges); then consume(t, ps) finishes (scale/bias/...). Messages are
    gathered per (tile, 32k-row table chunk) with dma_gather."""
    sb, boff, mbar, off = bmeta["sb"], bmeta["boff"], bmeta["mbar"], bmeta["off"]
    NQ = sb.shape[1]
    mbmax = max(1, max(mbar))
    for t in range(T):
        mb = mbar[t]
        if mb == 0:
            continue  # window tail beyond N: no nodes, nothing to write
        msg = pool.tile([P, mbmax * P], TBL_DT, tag="msg", bufs=4)
        col = 0
        for q in range(NQ):
            nb = int(sb[t][q])
            if nb == 0:
                continue
            base = q * CHK
            rows = min(CHK, TROWS - base)
            nc.gpsimd.dma_gather(
                out_ap=msg[:, col * P: (col + nb) * P]
                .rearrange("p (b f) -> p b f", f=P),
                in_ap=table[base: base + rows, :],
                idxs_ap=IDX_sb[:, boff[t][q] * 8: (boff[t][q] + nb) * 8],
                num_idxs=nb * P, num_idxs_reg=nb * P, elem_size=P,
                queue_num=_next_q())
            col += nb
        a01 = pool.tile([P, mbmax * P], TBL_DT, tag="a01", bufs=4)
        nc.vector.tensor_tensor(
            out=a01[:, : mb * P].rearrange("p (k f) -> p k f", f=P),
            in0=DSTL_sb[:, off[t]: off[t] + mb].to_broadcast([P, mb, P]),
            in1=iota_sb[:].unsqueeze(1).broadcast_to([P, mb, P]),
            op=mybir.AluOpType.is_equal,
        )
        ps = psum.tile([P, P], F32, tag="ps")
        for k in range(mb):
            nc.tensor.matmul(ps[:], lhsT=a01[:, k * P: (k + 1) * P],
                             rhs=msg[:, k * P: (k + 1) * P],
                             start=(k == 0), stop=(k == mb - 1))
        consume(t, ps)


def build_merged(pp, phases="ALL"):
    T, TO, G_LOC = pp["T"], pp["TO"], pp["G_LOC"]
    GTOT, B_PAD, NQ = pp["GTOT"], pp["B_PAD"], pp["NQ"]
    br = pp["branches"]
    nc = _new_nc()

    x0T_own = nc.dram_tensor("x0T_own", [P, TO * P], F32, kind="ExternalInput")
    iota = nc.dram_tensor("iota", [P, P], F32, kind="ExternalInput")
    ident = nc.dram_tensor("ident", [P, P], F32, kind="ExternalInput")
    ones_row = nc.dram_tensor("ones_row", [1, P], F32, kind="ExternalInput")
    rootx0T = nc.dram_tensor("rootx0T", [P, B_PAD], F32, kind="ExternalInput")
    NG = nc.dram_tensor("NG", [P, TO * 8], I16, kind="ExternalInput")
    GL = nc.dram_tensor("GL", [P, T], F32, kind="ExternalInput")
    GLR = nc.dram_tensor("GLR", [P, T], F32, kind="ExternalInput")
    cinv = nc.dram_tensor("cinv", [P, 1], F32, kind="ExternalInput")
    mlp_w1 = nc.dram_tensor("mlp_w1", [4 * P, 2 * P], F32, kind="ExternalInput")
    mlp_b1 = nc.dram_tensor("mlp_b1", [1, 2 * P], F32, kind="ExternalInput")
    mlp_w2 = nc.dram_tensor("mlp_w2", [2 * P, 2], F32, kind="ExternalInput")
    mlp_b2 = nc.dram_tensor("mlp_b2", [1, 2], F32, kind="ExternalInput")
    out = nc.dram_tensor("out", [P, 2], F32, kind="ExternalOutput")

    ins = {}
    for b in ("td", "bu"):
        M = br[b]["Mbar"]
        ins[b] = {
            "w1": nc.dram_tensor(f"w1{b}", [P, P], F32, kind="ExternalInput"),
            "b1": nc.dram_tensor(f"b1{b}", [1, P], F32, kind="ExternalInput"),
            "w2a": nc.dram_tensor(f"w2a{b}", [P, P], F32, kind="ExternalInput"),
            "w2b": nc.dram_tensor(f"w2b{b}", [P, P], F32, kind="ExternalInput"),
            "b2": nc.dram_tensor(f"b2{b}", [1, P], F32, kind="ExternalInput"),
            "dinv_own": nc.dram_tensor(f"dvo{b}", [P, TO], F32, kind="ExternalInput"),
            "dinv_loc": nc.dram_tensor(f"dvl{b}", [P, T], F32, kind="ExternalInput"),
            "IDX": nc.dram_tensor(f"IDX{b}", [P, M * 8], I16, kind="ExternalInput"),
            "DSTL": nc.dram_tensor(f"DSTL{b}", [P, M], F32, kind="ExternalInput"),
            "ht1": nc.dram_tensor(f"ht1{b}", [TO * P, P], TBL_DT, kind="Internal"),
            "TAB1": nc.dram_tensor(f"TAB1{b}", [GTOT, P], TBL_DT, kind="Internal"),
            "rtab": nc.dram_tensor(f"rtab{b}", [B_PAD, P], F32, kind="Internal"),
            "ht2": nc.dram_tensor(f"ht2{b}", [TO * P, P], TBL_DT, kind="Internal"),
            "TAB2": nc.dram_tensor(f"TAB2{b}", [GTOT, P], TBL_DT, kind="Internal"),
        }

    with tile.TileContext(nc) as tc:
        with (
            tc.tile_pool(name="sbuf", bufs=2) as pool,
            tc.tile_pool(name="cst", bufs=1) as cst,
            tc.tile_pool(name="psum", bufs=3, space="PSUM") as psum,
            tc.tile_pool(name="pps", bufs=2, space="PSUM") as pool_ps,
        ):
            iota_sb = _load(nc, cst, iota[:], (P, P), F32, "iota")
            ident_sb = _load(nc, cst, ident[:], (P, P), F32, "ident")
            ones_sb = _load(nc, cst, ones_row[:], (1, P), F32, "ones")
            NG_sb = _load(nc, cst, NG[:], (P, TO * 8), I16, "NG")
            GL_sb = _load(nc, cst, GL[:], (P, T), F32, "GL")
            GLR_sb = _load(nc, cst, GLR[:], (P, T), F32, "GLR")
            cinv_sb = _load(nc, cst, cinv[:], (P, 1), F32, "cinv")
            cs = {}
            for b in ("td", "bu"):
                cs[b] = {k: _load(nc, cst, ins[b][k][:],
                                  (P, P) if k in ("w1", "w2a", "w2b")
                                  else ((P, TO) if k == "dinv_own" else (P, T)),
                                  F32, f"{k}{b}")
                         for k in ("w1", "w2a", "w2b", "dinv_own", "dinv_loc")}

            # ---- phase A: owned ht1 tiles, both branches ----
            CH = 8
            for c0 in range(0, TO, CH):
                nb = min(CH, TO - c0)
                xt = pool.tile([P, CH * P], F32, tag="xt", bufs=2)
                nc.sync.dma_start(out=xt[:, : nb * P],
                                  in_=x0T_own[:, c0 * P: (c0 + nb) * P])
                st = {b: pool.tile([P, CH * P], TBL_DT, tag=f"st{b}", bufs=2,
                                   name=f"st{b}")
                      for b in ("td", "bu")}
                for j in range(nb):
                    for b in ("td", "bu"):
                        psx = psum.tile([P, P], F32, tag="ps")
                        nc.tensor.matmul(psx[:], lhsT=xt[:, j * P: (j + 1) * P],
                                         rhs=cs[b]["w1"][:], start=True, stop=True)
                        nc.vector.tensor_scalar(
                            out=st[b][:, j * P: (j + 1) * P], in0=psx[:],
                            scalar1=cs[b]["dinv_own"][:, c0 + j: c0 + j + 1],
                            scalar2=None, op0=mybir.AluOpType.mult)
                for b in ("td", "bu"):
                    nc.sync.dma_start(
                        out=ins[b]["ht1"][c0 * P: (c0 + nb) * P, :]
                        .rearrange("(j p) f -> p j f", p=P),
                        in_=st[b][:, : nb * P].rearrange("p (j f) -> p j f", f=P))

            # ---- phase B: AllGather ht1 -> TAB1 ----
            for b in ("td", "bu"):
                nc.gpsimd.collective_compute(
                    "AllGather", mybir.AluOpType.bypass,
                    replica_groups=[list(range(N_CORES))],
                    ins=[ins[b]["ht1"][:]], outs=[ins[b]["TAB1"][:]])

            if phases == "AB":
                nc.gpsimd.dma_start(out=out[:, :], in_=ins["td"]["TAB1"][0:P, 0:2])
            # ---- phase C: root tables R_b = relu(x0[roots]) @ W2b_b ----
            for j in range(B_PAD // P):
                rx = pool.tile([P, P], F32, tag="rx", bufs=2)
                nc.sync.dma_start(out=rx[:], in_=rootx0T[:, j * P: (j + 1) * P])
                rr = pool.tile([P, P], F32, tag="rr", bufs=2)
                nc.scalar.activation(out=rr[:], in_=rx[:],
                                     func=mybir.ActivationFunctionType.Relu)
                for b in ("td", "bu"):
                    psr = psum.tile([P, P], F32, tag="ps")
                    nc.tensor.matmul(psr[:], lhsT=rr[:], rhs=cs[b]["w2b"][:],
                                     start=True, stop=True)
                    ro = pool.tile([P, P], F32, tag="ro", bufs=2)
                    nc.vector.tensor_copy(out=ro[:], in_=psr[:])
                    nc.sync.dma_start(out=ins[b]["rtab"][j * P: (j + 1) * P, :],
                                      in_=ro[:])

            # ---- phase D: layer-1 aggregation, fused ht2-tile build ----
            rx2 = {}
            for b in ("td", "bu"):
                ib = ins[b]
                M = br[b]["Mbar"]
                IDX_sb = _load(nc, pool, ib["IDX"][:], (P, M * 8), I16, "idx")
                DSTL_sb = _load(nc, pool, ib["DSTL"][:], (P, M), F32, "dstl")
                b1_sb = _load(nc, cst, ib["b1"][:], (1, P), F32, f"b1{b}")
                btile1 = _bias_tile(nc, pool, psum, ones_sb, b1_sb, f"bt1{b}")
                ps_root = pool_ps.tile([G_LOC, P], F32, tag="acc")
                t_last = max(t for t in range(T) if br[b]["mbar"][t] > 0)
                rg_all = pool.tile([P, TO * P], F32, tag="rga", bufs=1)
                for c0 in range(0, TO, 8):
                    nb = min(8, TO - c0)
                    nc.gpsimd.dma_gather(
                        out_ap=rg_all[:, c0 * P: (c0 + nb) * P]
                        .rearrange("p (b f) -> p b f", f=P),
                        in_ap=ib["rtab"][:, :],
                        idxs_ap=NG_sb[:, c0 * 8: (c0 + nb) * 8],
                        num_idxs=nb * P, num_idxs_reg=nb * P, elem_size=P,
                        queue_num=_next_q())

                def consume1(t, ps, b=b, ib=ib, btile1=btile1, ps_root=ps_root,
                             t_last=t_last, rg_all=rg_all):
                    dvl = cs[b]["dinv_loc"]
                    xo = pool.tile([P, P], F32, tag="xo", bufs=3)
                    nc.vector.tensor_scalar(
                        out=xo[:], in0=ps[:], scalar1=dvl[:, t: t + 1],
                        scalar2=None, op0=mybir.AluOpType.mult)
                    nc.vector.tensor_add(out=xo[:], in0=xo[:], in1=btile1[:])
                    # root-feature accumulation: ps_root += onehot(GLR)^T @ xo
                    ohr = pool.tile([P, G_LOC], F32, tag="ohr", bufs=3)
                    nc.vector.tensor_tensor(
                        out=ohr[:], in0=GLR_sb[:, t: t + 1].to_broadcast([P, G_LOC]),
                        in1=iota_sb[:, :G_LOC], op=mybir.AluOpType.is_equal)
                    nc.tensor.matmul(ps_root[:], lhsT=ohr[:], rhs=xo[:],
                                     start=(t == 0), stop=(t == t_last))
                    if t < TO:
                        # fused layer-2 table tile:
                        # ht2 = dinv * (relu(x2)^T W2a + R[batch])
                        pst = psum.tile([P, P], F32, tag="pst")
                        nc.tensor.transpose(out=pst[:], in_=xo[:],
                                            identity=ident_sb[:])
                        xr = pool.tile([P, P], F32, tag="xr", bufs=3)
                        nc.scalar.activation(out=xr[:], in_=pst[:],
                                             func=mybir.ActivationFunctionType.Relu)
                        ps2 = psum.tile([P, P], F32, tag="ps2", bufs=2)
                        nc.tensor.matmul(ps2[:], lhsT=xr[:], rhs=cs[b]["w2a"][:],
                                         start=True, stop=False)
                        nc.tensor.matmul(ps2[:], lhsT=ident_sb[:],
                                         rhs=rg_all[:, t * P: (t + 1) * P],
                                         start=False, stop=True)
                        st2 = pool.tile([P, P], TBL_DT, tag="st2", bufs=3)
                        nc.vector.tensor_scalar(
                            out=st2[:], in0=ps2[:], scalar1=dvl[:, t: t + 1],
                            scalar2=None, op0=mybir.AluOpType.mult)
                        nc.sync.dma_start(
                            out=ib["ht2"][t * P: (t + 1) * P, :], in_=st2[:])

                _aggregate(nc, pool, psum, ib["TAB1"], IDX_sb, DSTL_sb,
                           br[b], GTOT, iota_sb, T, consume1)

                # finish root rows: [G_LOC, P] -> [P, G_LOC]
                rootS = pool.tile([G_LOC, P], F32, tag=f"rootS{b}", bufs=1)
                nc.vector.tensor_copy(out=rootS[:], in_=ps_root[:])
                psT = psum.tile([P, G_LOC], F32, tag="pst")
                nc.tensor.transpose(out=psT[:], in_=rootS[:],
                                    identity=ident_sb[:G_LOC, :G_LOC])
                rx2_sb = pool.tile([P, G_LOC], F32, tag=f"rx2{b}", bufs=1)
                nc.vector.tensor_copy(out=rx2_sb[:], in_=psT[:])
                rx2[b] = rx2_sb

                # ---- phase E: AllGather ht2 -> TAB2 (overlaps next branch) ----
                nc.gpsimd.collective_compute(
                    "AllGather", mybir.AluOpType.bypass,
                    replica_groups=[list(range(N_CORES))],
                    ins=[ib["ht2"][:]], outs=[ib["TAB2"][:]])

            # ---- phase F: layer-2 aggregation + pooling ----
            pooled = {}
            for b in ("td", "bu"):
                ib = ins[b]
                M = br[b]["Mbar"]
                IDX_sb = _load(nc, pool, ib["IDX"][:], (P, M * 8), I16, "idx")
                DSTL_sb = _load(nc, pool, ib["DSTL"][:], (P, M), F32, "dstl")
                b2_sb = _load(nc, cst, ib["b2"][:], (1, P), F32, f"b2{b}")
                btile2 = _bias_tile(nc, pool, psum, ones_sb, b2_sb, f"bt2{b}")
                ps_pool = pool_ps.tile([G_LOC, P], F32, tag="acc")
                t_last = max(t for t in range(T) if br[b]["mbar"][t] > 0)

                def consume2(t, ps, b=b, btile2=btile2, ps_pool=ps_pool,
                             t_last=t_last):
                    hs = pool.tile([P, P], F32, tag="hs", bufs=3)
                    nc.vector.tensor_scalar(
                        out=hs[:], in0=ps[:], scalar1=cs[b]["dinv_loc"][:, t: t + 1],
                        scalar2=None, op0=mybir.AluOpType.mult)
                    nc.vector.tensor_add(out=hs[:], in0=hs[:], in1=btile2[:])
                    h2 = pool.tile([P, P], F32, tag="h2", bufs=3)
                    nc.scalar.activation(out=h2[:], in_=hs[:],
                                         func=mybir.ActivationFunctionType.Relu)
                    oh = pool.tile([P, G_LOC], F32, tag="oh", bufs=3)
                    nc.vector.tensor_tensor(
                        out=oh[:], in0=GL_sb[:, t: t + 1].to_broadcast([P, G_LOC]),
                        in1=iota_sb[:, :G_LOC], op=mybir.AluOpType.is_equal)
                    nc.tensor.matmul(ps_pool[:], lhsT=oh[:], rhs=h2[:],
                                     start=(t == 0), stop=(t == t_last))

                _aggregate(nc, pool, psum, ib["TAB2"], IDX_sb, DSTL_sb,
                           br[b], GTOT, iota_sb, T, consume2)

                meanS = pool.tile([G_LOC, P], F32, tag=f"mean{b}", bufs=1)
                nc.vector.tensor_scalar(
                    out=meanS[:], in0=ps_pool[:], scalar1=cinv_sb[:G_LOC, :],
                    scalar2=None, op0=mybir.AluOpType.mult)
                pst = psum.tile([P, G_LOC], F32, tag="pst")
                nc.tensor.transpose(out=pst[:], in_=meanS[:],
                                    identity=ident_sb[:G_LOC, :G_LOC])
                meanT = pool.tile([P, G_LOC], F32, tag=f"meanT{b}", bufs=1)
                nc.vector.tensor_copy(out=meanT[:], in_=pst[:])
                pooled[b] = meanT

            # ---- phase G: final MLP over this core's G_LOC graph slots ----
            w1c_sb = cst.tile([P, 4 * 2 * P], F32, tag="mw1", name="mw1")
            nc.sync.dma_start(
                out=w1c_sb[:].rearrange("p (c o) -> p c o", c=4),
                in_=mlp_w1[:].rearrange("(c p) o -> p c o", p=P))
            b1m_sb = _load(nc, cst, mlp_b1[:], (1, 2 * P), F32, "mb1")
            w2c_sb = cst.tile([P, 2 * 2], F32, tag="mw2", name="mw2")
            nc.sync.dma_start(
                out=w2c_sb[:].rearrange("p (c o) -> p c o", c=2),
                in_=mlp_w2[:].rearrange("(c p) o -> p c o", p=P))
            b2m_sb = _load(nc, cst, mlp_b2[:], (1, 2), F32, "mb2")

            ps1 = psum.tile([G_LOC, 2 * P], F32, tag="ps")
            chunks = [pooled["bu"][:, :G_LOC], rx2["bu"][:, :G_LOC],
                      pooled["td"][:, :G_LOC], rx2["td"][:, :G_LOC]]
            for ci, lhsT in enumerate(chunks):
                nc.tensor.matmul(ps1[:], lhsT=lhsT,
                                 rhs=w1c_sb[:, ci * 2 * P: (ci + 1) * 2 * P],
                                 start=(ci == 0), stop=False)
            nc.tensor.matmul(ps1[:], lhsT=ones_sb[0:1, :G_LOC], rhs=b1m_sb[0:1, :],
                             start=False, stop=True)
            h1 = pool.tile([G_LOC, 2 * P], F32, tag="mlph", bufs=1)
            nc.scalar.activation(out=h1[:], in_=ps1[:],
                                 func=mybir.ActivationFunctionType.Relu)
            hT = []
            for ci in range(2):
                pst2 = psum.tile([P, G_LOC], F32, tag="pst")
                nc.tensor.transpose(out=pst2[:], in_=h1[:, ci * P: (ci + 1) * P],
                                    identity=ident_sb[:G_LOC, :G_LOC])
                ht_sb = pool.tile([P, G_LOC], F32, tag=f"hT{ci}", bufs=1)
                nc.vector.tensor_copy(out=ht_sb[:], in_=pst2[:])
                hT.append(ht_sb)
            ps2 = psum.tile([G_LOC, 2], F32, tag="ps")
            for ci in range(2):
                nc.tensor.matmul(ps2[:], lhsT=hT[ci][:, :G_LOC],
                                 rhs=w2c_sb[:, ci * 2: (ci + 1) * 2],
                                 start=(ci == 0), stop=False)
            nc.tensor.matmul(ps2[:], lhsT=ones_sb[0:1, :G_LOC], rhs=b2m_sb[0:1, :],
                             start=False, stop=True)
            oo = pool.tile([G_LOC, 2], F32, tag="oo", bufs=1)
            nc.vector.tensor_copy(out=oo[:], in_=ps2[:])
            nc.sync.dma_start(out=out[:G_LOC, :], in_=oo[:])
    nc.compile()
    return nc


# ----------------------------------------------------------------------------
# in_map assembly + kernel entry
# ----------------------------------------------------------------------------

def in_maps(pp, w):
    br = pp["branches"]
    ones = np.ones((1, P), np.float32)
    maps = []
    for c in range(N_CORES):
        m = {"x0T_own": pp["x0T_own"][c], "iota": pp["iota"],
             "ident": pp["ident"], "ones_row": ones,
             "rootx0T": pp["rootx0T"], "NG": pp["NG"][c],
             "GL": pp["GL"][c], "GLR": pp["GLR"][c], "cinv": pp["cinv"][c],
             "mlp_w1": w["mlp_w1"].astype(np.float32),
             "mlp_b1": w["mlp_b1"].astype(np.float32).reshape(1, -1),
             "mlp_w2": w["mlp_w2"].astype(np.float32),
             "mlp_b2": w["mlp_b2"].astype(np.float32).reshape(1, -1)}
        for b in ("td", "bu"):
            bb = br[b]
            m[f"w1{b}"] = np.ascontiguousarray(w[f"{b}_w1"].astype(np.float32))
            m[f"b1{b}"] = w[f"{b}_b1"].astype(np.float32).reshape(1, P)
            m[f"w2a{b}"] = np.ascontiguousarray(w[f"{b}_w2"][:P].astype(np.float32))
            m[f"w2b{b}"] = np.ascontiguousarray(w[f"{b}_w2"][P:].astype(np.float32))
            m[f"b2{b}"] = w[f"{b}_b2"].astype(np.float32).reshape(1, P)
            m[f"dvo{b}"] = bb["dinv_own"][c]
            m[f"dvl{b}"] = bb["dinv_loc"][c]
            m[f"IDX{b}"] = bb["packed"][c]["IDX16"]
            m[f"DSTL{b}"] = bb["packed"][c]["DSTL"]
        maps.append(m)
    return maps


def assemble_out(pp, results):
    B = pp["B"]
    out = np.zeros((B, 2), np.float32)
    for c in range(N_CORES):
        a, b = pp["g0"][c], pp["g0"][c + 1]
        out[a:b] = results[c]["out"][: b - a]
    return out


def kernel(x, x_da, edge_index, batch, rootindex,
           td_w1, td_b1, td_w2, td_b2,
           bu_w1, bu_b1, bu_w2, bu_b2,
           mlp_w1, mlp_b1, mlp_w2, mlp_b2):
    w = {"td_w1": td_w1, "td_b1": td_b1, "td_w2": td_w2, "td_b2": td_b2,
         "bu_w1": bu_w1, "bu_b1": bu_b1, "bu_w2": bu_w2, "bu_b2": bu_b2,
         "mlp_w1": mlp_w1, "mlp_b1": mlp_b1, "mlp_w2": mlp_w2, "mlp_b2": mlp_b2}
    w = {k: np.asarray(v) for k, v in w.items()}
    pp = preprocess(np.asarray(x), np.asarray(x_da), np.asarray(edge_index),
                    np.asarray(batch), np.asarray(rootindex))
    nc = build_merged(pp)
    res = run_bass_kernel_spmd(nc, in_maps(pp, w),
                               core_ids=list(range(N_CORES))).results
    return assemble_out(pp, res)
